# revision 2
# baseline (speedup 1.0000x reference)
"""InteractionNetwork (GNN message passing) on 8 trn2 NeuronCores.

Strategy: edges are sorted by receiver and sharded so that core i owns all
edges whose receiver falls in node range [i*1250, (i+1)*1250).  Each core:
  - runs the edge MLP (bf16 matmuls, fp32 PSUM accumulate) on its edges,
  - scatter-adds updated edge features into its node range via one-hot
    matmuls on the tensor engine (edges sorted => per-128-node-chunk PSUM
    accumulation),
  - runs the node MLP on its 1250 nodes.
No collective is needed: node ranges are disjoint.  The host gathers /
transposes / pads the inputs and un-permutes the outputs.
"""

import sys

sys.path.insert(0, "/opt/trn_rl_repo")

import numpy as np
import ml_dtypes

import concourse.bass as bass
import concourse.mybir as mybir
import concourse.tile as tile
from concourse import bacc
from concourse.bass_utils import run_bass_kernel_spmd

BF16 = ml_dtypes.bfloat16

N_NODES = 10000
N_EDGES = 320000
D = 128
H = 512
NCORES = 8
NPR = N_NODES // NCORES  # 1250 nodes per core
NCHUNK = 10  # ceil(1250/128) node chunks per core
NLOC = NCHUNK * 128  # 1280 local node rows
NODE_TILES = 3  # 1536 = 3*512 padded local nodes
P = 128
ET = 512  # edges per L1 tile (4 subtiles of 128)

F32 = mybir.dt.float32
BF = mybir.dt.bfloat16


def _build_program(NS, relu_act_chunks=(0,), uea_f32=True):
    """Build the (single, shared-across-cores) Bass/Tile program.

    NS: list of 10 ints, subtiles per node chunk (same for every core).
    """
    NSUM = int(sum(NS))
    assert NSUM % 4 == 0
    NT = NSUM // 4
    chunk_of = np.repeat(np.arange(NCHUNK), NS)  # subtile -> chunk
    first_sub = np.concatenate([[0], np.cumsum(NS)[:-1]])
    last_sub = np.cumsum(NS) - 1

    nc = bacc.Bacc(
        "TRN2",
        target_bir_lowering=False,
        debug=False,
        enable_asserts=True,
        num_devices=NCORES,
    )

    # ---- I/O ----
    in_t = nc.dram_tensor("in_t", [NT, 3, P, ET], BF, kind="ExternalInput").ap()
    oh_t = nc.dram_tensor("oh", [NSUM, P, P], BF, kind="ExternalInput").ap()
    we1_t = nc.dram_tensor("we1", [3, 4, P, P], BF, kind="ExternalInput").ap()
    we2_t = nc.dram_tensor("we2", [4, P, P], BF, kind="ExternalInput").ap()
    wn1_t = nc.dram_tensor("wn1", [2, 4, P, P], BF, kind="ExternalInput").ap()
    wn2_t = nc.dram_tensor("wn2", [4, P, P], BF, kind="ExternalInput").ap()
    be1_t = nc.dram_tensor("be1", [P, 4], F32, kind="ExternalInput").ap()
    bn1_t = nc.dram_tensor("bn1", [P, 4], F32, kind="ExternalInput").ap()
    xt_t = nc.dram_tensor("xt", [P, NODE_TILES * ET], BF, kind="ExternalInput").ap()
    uea_t = nc.dram_tensor(
        "uea", [NSUM * P, P], F32 if uea_f32 else BF, kind="ExternalOutput"
    ).ap()
    nodes_t = nc.dram_tensor("nodes", [NLOC, P], F32, kind="ExternalOutput").ap()

    from contextlib import ExitStack

    with tile.TileContext(nc) as tc, ExitStack() as stack:
        cpool = stack.enter_context(tc.tile_pool(name="consts", bufs=1))
        iopool = stack.enter_context(tc.tile_pool(name="io", bufs=3))
        wpool = stack.enter_context(tc.tile_pool(name="work", bufs=4))
        ps_h = stack.enter_context(tc.tile_pool(name="ps_h", bufs=2, space="PSUM"))
        ps_u = stack.enter_context(tc.tile_pool(name="ps_u", bufs=2, space="PSUM"))
        ps_a = stack.enter_context(tc.tile_pool(name="ps_a", bufs=2, space="PSUM"))
        ps_t = stack.enter_context(tc.tile_pool(name="ps_t", bufs=2, space="PSUM"))
        if True:
            # ---- constants to SBUF ----
            we1_sb = cpool.tile([P, 3, 4, P], BF)
            for kc in range(3):
                for hc in range(4):
                    nc.sync.dma_start(we1_sb[:, kc, hc], we1_t[kc, hc])
            we2_sb = cpool.tile([P, 4, P], BF)
            for hc in range(4):
                nc.sync.dma_start(we2_sb[:, hc], we2_t[hc])
            wn1_sb = cpool.tile([P, 2, 4, P], BF)
            for kc in range(2):
                for hc in range(4):
                    nc.sync.dma_start(wn1_sb[:, kc, hc], wn1_t[kc, hc])
            wn2_sb = cpool.tile([P, 4, P], BF)
            for hc in range(4):
                nc.sync.dma_start(wn2_sb[:, hc], wn2_t[hc])
            be1_sb = cpool.tile([P, 4], F32)
            nc.sync.dma_start(be1_sb[:], be1_t[:])
            bn1_sb = cpool.tile([P, 4], F32)
            nc.sync.dma_start(bn1_sb[:], bn1_t[:])
            xt_sb = cpool.tile([P, NODE_TILES * ET], BF)
            nc.sync.dma_start(xt_sb[:], xt_t[:])
            ident_sb = cpool.tile([P, P], BF)
            from concourse.masks import make_identity

            make_identity(nc, ident_sb[:])

            agg_sb = cpool.tile([P, NCHUNK, P], F32)
            aggT_sb = cpool.tile([P, NODE_TILES * ET], BF)
            nc.vector.memset(aggT_sb[:], 0.0)

            # ---- edge pipeline ----
            pa = None
            cur_chunk = -1
            for t in range(NT):
                in_sb = iopool.tile([P, 3, ET], BF, tag="in")
                for kc in range(3):
                    nc.sync.dma_start(in_sb[:, kc], in_t[t, kc])
                h1_sb = wpool.tile([P, 4, ET], BF, tag="h1")
                for hc in range(4):
                    ph = ps_h.tile([P, ET], F32, tag="h")
                    for kc in range(3):
                        nc.tensor.matmul(
                            ph[:],
                            we1_sb[:, kc, hc],
                            in_sb[:, kc],
                            start=(kc == 0),
                            stop=(kc == 2),
                        )
                    # relu(+bias) PSUM->SBUF bf16, split across ACT / DVE
                    if hc in relu_act_chunks:
                        nc.scalar.activation(
                            h1_sb[:, hc],
                            ph[:],
                            mybir.ActivationFunctionType.Relu,
                            bias=be1_sb[:, hc : hc + 1],
                        )
                    else:
                        nc.vector.tensor_scalar(
                            h1_sb[:, hc],
                            ph[:],
                            be1_sb[:, hc : hc + 1],
                            0.0,
                            mybir.AluOpType.add,
                            mybir.AluOpType.max,
                        )
                for se in range(4):
                    s = t * 4 + se
                    pu = ps_u.tile([P, P], F32, tag="u")
                    for hc in range(4):
                        nc.tensor.matmul(
                            pu[:],
                            h1_sb[:, hc, se * P : (se + 1) * P],
                            we2_sb[:, hc],
                            start=(hc == 0),
                            stop=(hc == 3),
                        )
                    # bf16 copy for the scatter matmul rhs
                    u_sb = wpool.tile([P, P], BF, tag="usb")
                    nc.vector.tensor_copy(u_sb[:], pu[:])
                    # f32 (or bf16) staging for DMA out
                    if uea_f32:
                        u32_sb = wpool.tile([P, P], F32, tag="u32")
                        nc.scalar.copy(u32_sb[:], pu[:])
                        nc.sync.dma_start(uea_t[s * P : (s + 1) * P], u32_sb[:])
                    else:
                        nc.sync.dma_start(uea_t[s * P : (s + 1) * P], u_sb[:])
                    # scatter-add via one-hot matmul
                    oh_sb = iopool.tile([P, P], BF, tag="oh")
                    nc.sync.dma_start(oh_sb[:], oh_t[s])
                    c = int(chunk_of[s])
                    if c != cur_chunk:
                        pa = ps_a.tile([P, P], F32, tag="agg")
                        cur_chunk = c
                    nc.tensor.matmul(
                        pa[:],
                        oh_sb[:],
                        u_sb[:],
                        start=(s == first_sub[c]),
                        stop=(s == last_sub[c]),
                        skip_group_check=True,
                    )
                    if s == last_sub[c]:
                        nc.vector.tensor_copy(agg_sb[:, c], pa[:])

            # ---- aggregate transpose: agg [n,d] -> aggT [d,n] (bf16) ----
            aggb_sb = cpool.tile([P, NCHUNK, P], BF)
            nc.vector.tensor_copy(aggb_sb[:], agg_sb[:])
            for c in range(NCHUNK):
                pt = ps_t.tile([P, P], BF, tag="tr")
                nc.tensor.transpose(pt[:], aggb_sb[:, c], ident_sb[:])
                nc.scalar.copy(aggT_sb[:, c * P : (c + 1) * P], pt[:])

            # ---- node MLP ----
            for nt_i in range(NODE_TILES):
                sl = slice(nt_i * ET, (nt_i + 1) * ET)
                h1n_sb = wpool.tile([P, 4, ET], BF, tag="h1")
                for hc in range(4):
                    ph = ps_h.tile([P, ET], F32, tag="h")
                    nc.tensor.matmul(
                        ph[:], wn1_sb[:, 0, hc], xt_sb[:, sl], start=True, stop=False
                    )
                    nc.tensor.matmul(
                        ph[:], wn1_sb[:, 1, hc], aggT_sb[:, sl], start=False, stop=True
                    )
                    nc.scalar.activation(
                        h1n_sb[:, hc],
                        ph[:],
                        mybir.ActivationFunctionType.Relu,
                        bias=bn1_sb[:, hc : hc + 1],
                    )
                for sn in range(4):
                    gi = nt_i * 4 + sn
                    if gi >= NCHUNK:
                        continue
                    pu = ps_u.tile([P, P], F32, tag="u")
                    for hc in range(4):
                        nc.tensor.matmul(
                            pu[:],
                            h1n_sb[:, hc, sn * P : (sn + 1) * P],
                            wn2_sb[:, hc],
                            start=(hc == 0),
                            stop=(hc == 3),
                        )
                    n32_sb = wpool.tile([P, P], F32, tag="u32")
                    nc.scalar.copy(n32_sb[:], pu[:])
                    nc.sync.dma_start(nodes_t[gi * P : (gi + 1) * P], n32_sb[:])

    nc.compile()
    return nc


def _prep_inputs(x, edge_index, edge_attr, We1, be1, We2, be2, Wn1, bn1, Wn2, bn2):
    """Host-side shard / sort / gather / transpose.  Returns (in_maps, meta)."""
    x = np.asarray(x, np.float32)
    edge_attr = np.asarray(edge_attr, np.float32)
    send = np.asarray(edge_index[0], np.int64)
    recv = np.asarray(edge_index[1], np.int64)

    core = recv // NPR
    local = recv - core * NPR
    chunk = local // P  # 0..9

    # fold nonzero be2/bn2 into We2/Wn2 via... (biases are zero in this
    # problem; if not, add them on host to the output is impossible --
    # instead fall back to appending bias via an extra hidden unit).
    We1 = np.asarray(We1, np.float32)
    We2 = np.asarray(We2, np.float32)
    Wn1 = np.asarray(Wn1, np.float32)
    Wn2 = np.asarray(Wn2, np.float32)
    be1 = np.asarray(be1, np.float32)
    bn1 = np.asarray(bn1, np.float32)
    be2 = np.asarray(be2, np.float32)
    bn2 = np.asarray(bn2, np.float32)

    # per (core, chunk) edge lists
    counts = np.zeros((NCORES, NCHUNK), np.int64)
    np.add.at(counts, (core, chunk), 1)
    NS = np.maximum(1, np.ceil(counts / P).astype(np.int64).max(axis=0))
    NS[-1] += (-NS.sum()) % 4
    NS = [int(v) for v in NS]
    NSUM = int(sum(NS))
    NT = NSUM // 4
    sbase = np.concatenate([[0], np.cumsum(NS)[:-1]]) * P  # slot base per chunk

    order = np.argsort(core * NLOC + local, kind="stable")

    s_arr = np.arange(NSUM * P) // P
    chunkmap = np.repeat(np.arange(NCHUNK), NS)

    # shared weight tensors
    we1_tiles = np.ascontiguousarray(
        We1.reshape(3, P, 4, P).transpose(0, 2, 1, 3)
    ).astype(BF16)
    we2_tiles = np.ascontiguousarray(We2.reshape(4, P, P)).astype(BF16)
    wn1_tiles = np.ascontiguousarray(
        Wn1.reshape(2, P, 4, P).transpose(0, 2, 1, 3)
    ).astype(BF16)
    wn2_tiles = np.ascontiguousarray(Wn2.reshape(4, P, P)).astype(BF16)
    be1_cols = np.ascontiguousarray(be1.reshape(4, P).T)
    bn1_cols = np.ascontiguousarray(bn1.reshape(4, P).T)

    in_maps = []
    meta = []  # per-core (ids, valid) for output unsort
    L = NSUM * P
    for i in range(NCORES):
        ids_i = order[core[order] == i]  # sorted by (chunk, orig order)
        ch_i = chunk[ids_i]
        eidx = np.full(L, -1, np.int64)
        for c in range(NCHUNK):
            sel = ids_i[ch_i == c]
            eidx[sbase[c] : sbase[c] + len(sel)] = sel
        valid = eidx >= 0
        ids = eidx[valid]

        inp = np.zeros((L, 3 * P), np.float32)
        inp[valid, 0:P] = x[send[ids]]
        inp[valid, P : 2 * P] = x[recv[ids]]
        inp[valid, 2 * P : 3 * P] = edge_attr[ids]
        in_tiles = np.ascontiguousarray(
            inp.T.reshape(3, P, NT, ET).transpose(2, 0, 1, 3)
        ).astype(BF16)

        oh = np.zeros((NSUM, P, P), np.float32)
        rows = np.arange(L) % P
        cols = np.zeros(L, np.int64)
        cols[valid] = local[ids] - P * chunkmap[s_arr[valid]]
        oh[s_arr[valid], rows[valid], cols[valid]] = 1.0
        oh = oh.astype(BF16)

        xt = np.zeros((P, NODE_TILES * ET), np.float32)
        lo = i * NPR
        n_have = min(NLOC, N_NODES - lo)
        xt[:, :n_have] = x[lo : lo + n_have].T
        xt = xt.astype(BF16)

        in_maps.append(
            {
                "in_t": in_tiles,
                "oh": oh,
                "we1": we1_tiles,
                "we2": we2_tiles,
                "wn1": wn1_tiles,
                "wn2": wn2_tiles,
                "be1": be1_cols,
                "bn1": bn1_cols,
                "xt": xt,
            }
        )
        meta.append((ids, valid))

    if np.any(be2) or np.any(bn2):
        raise NotImplementedError(
            "nonzero second-layer biases not supported by this kernel"
        )

    return in_maps, meta, NS


def kernel(x, edge_index, edge_attr, We1, be1, We2, be2, Wn1, bn1, Wn2, bn2):
    in_maps, meta, NS = _prep_inputs(
        x, edge_index, edge_attr, We1, be1, We2, be2, Wn1, bn1, Wn2, bn2
    )
    nc = _build_program(NS)
    res = run_bass_kernel_spmd(nc, in_maps, core_ids=list(range(NCORES)))
    results = res.results

    uea_full = np.zeros((N_EDGES, D), np.float32)
    nodes_full = np.zeros((N_NODES, D), np.float32)
    for i in range(NCORES):
        ids, valid = meta[i]
        uea_full[ids] = np.asarray(results[i]["uea"], np.float32)[valid]
        nodes_full[i * NPR : (i + 1) * NPR] = np.asarray(
            results[i]["nodes"], np.float32
        )[:NPR]
    return nodes_full, uea_full
